# revision 27
# baseline (speedup 1.0000x reference)
"""Trainium2 Bass kernel for the attention-pooling module.

Reference math (B=32, N=2048, D=512, K=256):
    vIp   = vI @ Wi                                   [B,N,K]
    vQp   = vQ @ Wq + bq                              [B,K]
    ha    = leaky_relu(vIp + vQp[:,None,:], 0.01)     [B,N,K]
    scores= ha @ Wp[:,0] + bp                         [B,N]   (bp shift cancels in softmax)
    pi    = softmax(scores, -1)                       [B,N]
    out   = einsum("bn,bnk->bk", pi, vIp) + vQp       [B,K]

Kernel strategy (8 cores, data-parallel over B, 4 batches/core):
  - vI streams once per layout as fp8-e4m3 (host-cast): viT [d-part, n] for
    the vIp^T matmuls and vnat [n-part, d] for the u = e @ vI contraction,
    packed into ONE 2 MiB DMA per batch so the Sync queue issues 8 input
    DMAs total instead of ~30 (issue costs ~1 us each).
  - vIpT = Wi.T @ vIT in [K-part, N-free] layout (fp8 DoubleRow matmuls);
    ha = ACT Lrelu(vIpT/16 + vQp_k) fused, emitted as [128,1024] tiles.
  - scores^T directly: per 128-wide n-tile, one DR matmul with lhsT = ha
    chunk (stationary) and rhs = the Wp column -> sc[128,16] PSUM columns.
    This kills the old row->column conversion (4 DVE row-copies + 2 DMA
    transposes per batch) that serialized the tail and let HAM throttle
    the PE clock between phases.
  - exp reads sc straight from PSUM -> e_col fp8 + zp accum in one ACT op;
    Z via a ones-column matmul; invz = DVE reciprocal of the PSUM scalar.
  - u = e @ vnat on the PE (8 DR matmuls); the PSUM->SBUF cast is fused
    with the 1/Z scale in one DVE tensor_scalar into a [4,512] U tile.
  - Final projection batched over all 4 local batches: 4 PE transposes of
    U -> U^T columns, one fp8 cast, 8 plain-fp8 matmuls att^T = Wi^T U^T,
    and a single DVE op out^T = att^T/16 + vQp^T.  Output leaves column-
    major [128, KC, BLOC]; the host transposes back.
"""

import os
import sys

sys.path.insert(0, "/opt/trn_rl_repo")

import numpy as np
import ml_dtypes

from concourse import bass, bacc, tile, mybir
from concourse.bass_utils import run_bass_kernel_spmd

dt = mybir.dt
F32, BF16, FP8 = dt.float32, dt.bfloat16, dt.float8e4
AF = mybir.ActivationFunctionType
ALU = mybir.AluOpType
DR = mybir.MatmulPerfMode.DoubleRow

B, N, D, K = 32, 2048, 512, 256
NCORES = 8
BLOC = B // NCORES           # 4 batches per core
SUP = 512                    # matmul free-dim supertile (PSUM-bank limited)
WSUP = 1024                  # ha / vp double-wide tile
DC = D // 128                # 4 contraction chunks
KC = K // 128                # 2 K chunks
NT = N // 128                # 16 n-tiles
NEG = 0.01

VIT_B = 4 * N                # fp8 bytes per partition of viT block
VIN_B = VIT_B + NT * D       # + vnat block


def build_nc():
    nc = bacc.Bacc("TRN2", target_bir_lowering=False, debug=False)

    vin_d = nc.dram_tensor("vin", [BLOC, 128, VIN_B], FP8, kind="ExternalInput")
    w8_d = nc.dram_tensor("w8", [128, 1056], FP8, kind="ExternalInput")
    pkh_d = nc.dram_tensor("pkh", [128, 1040], BF16, kind="ExternalInput")
    pkf_d = nc.dram_tensor("pkf", [128, 3], F32, kind="ExternalInput")
    outT_d = nc.dram_tensor("outT", [128, KC * BLOC], F32, kind="ExternalOutput")

    with tile.TileContext(nc) as tc:
        with (
            tc.tile_pool(name="const", bufs=1) as cpool,
            tc.tile_pool(name="stream", bufs=4) as spool,
            tc.tile_pool(name="work", bufs=2) as wpool,
            tc.tile_pool(name="pmm", bufs=3, space=bass.MemorySpace.PSUM) as pmm,
            tc.tile_pool(name="psm", bufs=2, space=bass.MemorySpace.PSUM) as psm,
        ):
            w8_sb = cpool.tile([128, 1056], FP8, tag="w8")
            pkh_sb = cpool.tile([128, 1040], BF16, tag="pkh")
            pkf_sb = cpool.tile([128, 3], F32, tag="pkf")
            one16_sb = cpool.tile([1, 1], BF16, tag="one16")

            vins = [
                spool.tile([128, VIN_B], FP8, tag="vin", name=f"vin{b}")
                for b in range(BLOC)
            ]

            def vit_view(b):
                return vins[b][:, 0:VIT_B].rearrange(
                    "p (cc i n) -> p cc i n", cc=2, i=2
                )

            def vnat_view(b):
                return vins[b][:, VIT_B:VIN_B].rearrange("p (t d) -> p t d", t=NT)

            # input DMAs, ordered so batch-0 compute can start earliest
            vin0_dview = vin_d[0][:, 0:VIT_B].rearrange(
                "p (cc i n) -> p cc i n", cc=2, i=2
            )
            # Everything rides the sync HWDGE ring, FIFO = wire priority.
            # (gpsimd dma_start = slow SWDGE; a second HWDGE ring would
            # round-robin packets and destroy the priority order.)
            nc.sync.dma_start(out=w8_sb[:], in_=w8_d[:])
            nc.sync.dma_start(out=pkh_sb[:], in_=pkh_d[:])
            nc.sync.dma_start(out=pkf_sb[:], in_=pkf_d[:])
            nc.sync.dma_start(
                out=vit_view(0)[:, :, :, 0:WSUP], in_=vin0_dview[:, :, :, 0:WSUP]
            )
            nc.sync.dma_start(
                out=vit_view(0)[:, :, :, WSUP:N], in_=vin0_dview[:, :, :, WSUP:N]
            )
            nc.sync.dma_start(out=vins[0][:, VIT_B:VIN_B], in_=vin_d[0][:, VIT_B:VIN_B])
            for b in range(1, BLOC):
                nc.sync.dma_start(out=vins[b][:], in_=vin_d[b])

            wq_sb = pkh_sb[:, 0:1024].rearrange("p (c k) -> p c k", c=DC)
            vqt_sb = pkh_sb[:, 1024:1040].rearrange("p (c b) -> p c b", c=DC)
            bq_sb = pkf_sb[:, 0:2]
            onesc_sb = pkf_sb[:, 2:3]
            wi8_sb = w8_sb[:, 0:1024].rearrange("p (cc i k) -> p cc i k", cc=2, i=2)
            wi8f_sb = w8_sb[:, 0:1024].rearrange("p (c k) -> p c k", c=DC)
            wp8_sb = w8_sb[:, 1024:1056].rearrange("p (i j) -> p i j", i=2)

            # one PSUM bank, multi-use: scores^T cols 0..63, z col 64+b,
            # vqp scratch cols 128.., utp cols 256.., atp cols 320..,
            # warmup cols 384..
            sc4 = psm.tile([128, 512], F32, tag="sc4", bufs=1)

            # ---- PE warm-up: dummy matmuls on a memset tile keep the HAM
            # clock gate at full speed while the first DMAs stream in ----
            wu_sb = cpool.tile([128, SUP], FP8, tag="wu")
            nc.vector.memset(wu_sb[:], 0.25)
            for _ in range(40):
                nc.tensor.matmul(
                    sc4[0:1, 384:512], wu_sb[:, 0:1], wu_sb[:, 0:128],
                    start=True, stop=True,
                )

            # ---- vQp^T[k, b] = sum_d Wq[d,k] vQ[b,d] + bq[k]  (K on part) ----
            vqpt_sb = cpool.tile([128, KC, BLOC], F32, tag="vqpt")
            for kc in range(KC):
                vqpt_ps = sc4[:, 128 : 128 + BLOC]
                for c in range(DC):
                    nc.tensor.matmul(
                        vqpt_ps[:],
                        wq_sb[:, c, kc * 128 : (kc + 1) * 128],
                        vqt_sb[:, c, :],
                        start=(c == 0),
                        stop=(c == DC - 1),
                    )
                nc.vector.tensor_scalar(
                    vqpt_sb[:, kc, :], vqpt_ps[:], bq_sb[:, kc : kc + 1], None, ALU.add
                )

            nc.vector.memset(one16_sb[:], 1.0)

            u16s = [
                spool.tile([1, D], BF16, tag="u16", name=f"u16_{b}")
                for b in range(BLOC)
            ]

            def phase_scores(b):
                vit = vit_view(b)
                # vIpT + ha, [128,1024] at a time
                ha = wpool.tile([128, KC, N], FP8, tag="ha", name=f"ha{b}")
                for sp in range(2):
                    for kc in range(KC):
                        vp = pmm.tile([128, WSUP], F32, tag="vp")
                        for h in range(2):
                            n0 = sp * WSUP + h * SUP
                            for cc in range(2):
                                nc.tensor.matmul(
                                    vp[:, h * SUP : (h + 1) * SUP],
                                    wi8_sb[:, cc, :, kc * 128 : (kc + 1) * 128],
                                    vit[:, cc, :, n0 : n0 + SUP],
                                    perf_mode=DR,
                                    start=(cc == 0),
                                    stop=(cc == 1),
                                )
                        # Wi host-scaled x16 into fp8 range; ACT de-scales
                        nc.scalar.activation(
                            ha[:, kc, sp * WSUP : (sp + 1) * WSUP], vp[:], AF.Lrelu,
                            bias=vqpt_sb[:, kc, b : b + 1], scale=1.0 / 16, alpha=NEG,
                        )
                # scores^T: plain-fp8 matmuls (FWL, 128-col weight loads) with
                # ha stationary; two accumulating matmuls per 128-wide n-tile
                for t in range(NT):
                    for kc in range(KC):
                        nc.tensor.matmul(
                            sc4[:, b * NT + t : b * NT + t + 1],
                            ha[:, kc, t * 128 : (t + 1) * 128],
                            wp8_sb[:, kc, 0:1],
                            start=(kc == 0),
                            stop=(kc == KC - 1),
                        )

            def phase_tail(b):
                # exp straight off PSUM; e pairs at +16B for the DR lhsT AP
                ecol = wpool.tile([128, 2, 16], FP8, tag="ecol", name=f"ecol{b}")
                zp = wpool.tile([128, 1], F32, tag="zp")
                nc.scalar.activation(
                    ecol[:].rearrange("p i j -> p j i")[:, 0:8, :],
                    sc4[:, b * NT : (b + 1) * NT].rearrange("p (j i) -> p j i", i=2),
                    AF.Exp, scale=1.0 / 8, accum_out=zp[:],
                )
                nc.tensor.matmul(
                    sc4[0:1, 64 + b : 65 + b], onesc_sb[:], zp[:],
                    start=True, stop=True,
                )
                invz = wpool.tile([1, 1], F32, tag="invz")
                nc.vector.reciprocal(invz[:], sc4[0:1, 64 + b : 65 + b])
                # u = e @ vI  (row form), then scale by 1/Z during the cast
                vnat = vnat_view(b)
                ups = pmm.tile([1, D], F32, tag="vp")
                for t in range(0, NT, 2):
                    nc.tensor.matmul(
                        ups[:],
                        ecol[:, :, t // 2 : t // 2 + 1],
                        vnat[:, t : t + 2, :],
                        perf_mode=DR,
                        start=(t == 0),
                        stop=(t == NT - 2),
                    )
                nc.vector.tensor_scalar(
                    u16s[b][:], ups[:], invz[:], None, ALU.mult
                )

            # Lrelu-block then Exp-block: the ACT engine swaps function
            # tables per switch (~1.3us each); grouping keeps it to one swap
            for b in range(BLOC):
                phase_scores(b)
            for b in range(BLOC):
                phase_tail(b)

            # ---- batched tail: att^T = Wi^T @ U^T, out^T = att^T/16 + vQp^T ----
            utp = pmm.tile([128, DC, BLOC, 2], BF16, tag="vp")
            for b in range(BLOC):
                for c in range(DC):
                    nc.tensor.transpose(
                        utp[:, c, b, 0:1],
                        u16s[b][0:1, c * 128 : (c + 1) * 128],
                        one16_sb[:],
                    )
            ut8 = wpool.tile([128, DC, BLOC], FP8, tag="ut8")
            nc.vector.tensor_copy(ut8[:], utp[:, :, :, 0])
            atp = pmm.tile([128, KC, BLOC], F32, tag="vp")
            for kc in range(KC):
                for c in range(DC):
                    nc.tensor.matmul(
                        atp[:, kc, :],
                        wi8f_sb[:, c, kc * 128 : (kc + 1) * 128],
                        ut8[:, c, :],
                        start=(c == 0),
                        stop=(c == DC - 1),
                    )
            outT_sb = cpool.tile([128, KC, BLOC], F32, tag="outT")
            nc.vector.scalar_tensor_tensor(
                outT_sb[:], atp[:], 1.0 / 16, vqpt_sb[:], ALU.mult, ALU.add
            )
            nc.sync.dma_start(
                out=outT_d[:, :], in_=outT_sb[:].rearrange("p a b -> p (a b)")
            )

    nc.compile()
    return nc


_NC = None


def _get_nc():
    global _NC
    if _NC is None:
        _NC = build_nc()
    return _NC


def kernel(vI, vQ, Wi, Wq, bq, Wp, bp, **_unused):
    vI = np.asarray(vI, dtype=np.float32)
    vQ = np.asarray(vQ, dtype=np.float32)
    Wi = np.asarray(Wi, dtype=np.float32)
    Wq = np.asarray(Wq, dtype=np.float32)
    bq = np.asarray(bq, dtype=np.float32)
    Wp = np.asarray(Wp, dtype=np.float32)
    # bp shifts every score equally -> cancels in softmax; ignored.

    bf = ml_dtypes.bfloat16
    f8 = ml_dtypes.float8_e4m3
    vi8 = vI.astype(f8)
    # viT, DoubleRow layout: d = cc*256 + i*128 + p  ->  [B, p, cc, i, N]
    viT = (
        vi8.transpose(0, 2, 1)
        .reshape(B, 2, 2, 128, N)
        .transpose(0, 3, 1, 2, 4)
        .reshape(B, 128, VIT_B)
    )
    vnat = vi8.reshape(B, NT, 128, D).transpose(0, 2, 1, 3).reshape(B, 128, NT * D)
    vin = np.ascontiguousarray(np.concatenate([viT, vnat], axis=2))

    wi8_dr = (
        (Wi * 16.0).reshape(2, 2, 128, K).transpose(2, 0, 1, 3).reshape(128, 1024)
    )
    wp_h = Wp[:, 0].reshape(KC, 128).T                           # [128,KC]
    wp_pad = np.zeros((128, 2, 16), np.float32)
    wp_pad[:, :, 0] = wp_h * 8.0
    w8 = np.ascontiguousarray(
        np.concatenate([wi8_dr, wp_pad.reshape(128, 32)], axis=1)
    ).astype(f8)

    wq_h = Wq.reshape(DC, 128, K).transpose(1, 0, 2).reshape(128, DC * K)
    bq_h = bq.reshape(KC, 128).T                                 # [128,KC]
    onesc = np.ones((128, 1), dtype=np.float32)

    pkf = np.ascontiguousarray(np.concatenate([bq_h, onesc], axis=1)).astype(
        np.float32
    )

    def pkh_for(core):
        vqc = vQ[core * BLOC : (core + 1) * BLOC]                # [BLOC, D]
        vqt = vqc.T.reshape(DC, 128, BLOC).transpose(1, 0, 2)    # [128,DC,BLOC]
        return np.ascontiguousarray(
            np.concatenate([wq_h, vqt.reshape(128, DC * BLOC)], axis=1)
        ).astype(bf)

    in_maps = []
    for c in range(NCORES):
        in_maps.append(
            {
                "vin": vin[c * BLOC : (c + 1) * BLOC],
                "w8": w8,
                "pkh": pkh_for(c),
                "pkf": pkf,
            }
        )

    nc = _get_nc()
    res = run_bass_kernel_spmd(
        nc, in_maps, list(range(NCORES)),
        trace=bool(int(os.environ.get("KERNEL_TRACE", "0"))),
        tmpdir=globals().get("TRACE_TMPDIR"),
    )
    kernel.last_results = res
    out = np.empty((B, K), dtype=np.float32)
    for c in range(NCORES):
        oT = res.results[c]["outT"].reshape(128, KC, BLOC)       # [p, kc, b]
        out[c * BLOC : (c + 1) * BLOC] = oT.transpose(2, 1, 0).reshape(BLOC, K)
    return out
